# revision 29
# baseline (speedup 1.0000x reference)
import numpy as np
from contextlib import ExitStack

import concourse.bass as bass
import concourse.tile as tile
from concourse import bacc, mybir
from concourse.bass_utils import run_bass_kernel_spmd

N, C, H, W = 256, 3, 256, 256
D = C * H * W          # 196608
NCORES = 8
RPC = N // NCORES      # 32 rows per core
Q = 4
P = 128
DPP = D // Q           # 49152 fp16 columns per partition
EPS = 1e-6

# Hybrid layout. Measured fp16 engine rates: DVE tensor_tensor (no accum)
# 0.5 cyc/elem, any accum-bearing DVE op 1.0 cyc/elem, ACT 1.0 cyc/elem
# @1.2GHz, PE matmul 1 moving-col/cyc @2.4GHz. Five reduction passes are
# needed (Sz,Sb,Szz,Sbb,Szb); DVE+ACT alone cannot cover them within the
# DMA window, so part of the data is packed "transposed" (T-layout: d on
# partitions, (block,row) on columns) and reduced on the idle TensorE
# with a ones-stationary matmul accumulating in PSUM.
#   R-segment (X cols):  DVE stt+acc z*b; ACT Sq z, Sq b, Copy z, Copy b
#   T-segment (Y cols):  DVE z*b, z*z, b*b at 2x; PE 5 streams -> PSUM
# Interleaved schedule: (kind, cols). Small leading chunks fill the
# pipeline fast; small trailing chunks cut the drain tail.
R_CHUNKS = [1536, 4608, 4608, 4608, 2560]
T_SUBS = [1024] + [2560] * 11 + [1536, 512]
SCHED = [("T", 0), ("R", 0), ("T", 1), ("T", 2), ("R", 1), ("T", 3),
         ("T", 4), ("T", 5), ("R", 2), ("T", 6), ("T", 7), ("T", 8),
         ("R", 3), ("T", 9), ("T", 10), ("T", 11), ("R", 4), ("T", 12),
         ("T", 13)]
X = sum(R_CHUNKS)      # 18432
Y = sum(T_SUBS)        # 30720
CRMAX = max(R_CHUNKS)
assert X + Y == DPP
D_R = 4 * X            # 73728 elements of each row in R layout
D_T = 4 * Y            # 122880 elements in T layout (960 blocks of 128)
NBLK = D_T // 128      # blocks per row
MMW = 512              # moving cols per matmul
NMM_STREAM = Y // MMW  # 60 matmuls per stream
NSTREAM = 5            # z, b, zb, zz, bb

_NC = None


def _build_nc():
    fp32 = mybir.dt.float32
    fp16 = mybir.dt.float16
    AF = mybir.ActivationFunctionType
    ALU = mybir.AluOpType
    AX = mybir.AxisListType

    nc = bacc.Bacc()
    z_ext = nc.dram_tensor("z", [P, DPP], fp16, kind="ExternalInput")
    b_ext = nc.dram_tensor("b", [P, DPP], fp16, kind="ExternalInput")
    out_ext = nc.dram_tensor("out", [P, 6], fp32, kind="ExternalOutput")
    out2_ext = nc.dram_tensor("out2", [1, NSTREAM * MMW], fp32,
                              kind="ExternalOutput")

    with tile.TileContext(nc) as tc, ExitStack() as ctx:
        zrp = ctx.enter_context(tc.tile_pool(name="zrp", bufs=3))
        brp = ctx.enter_context(tc.tile_pool(name="brp", bufs=3))
        ztp = ctx.enter_context(tc.tile_pool(name="ztp", bufs=3))
        btp = ctx.enter_context(tc.tile_pool(name="btp", bufs=3))
        pp = ctx.enter_context(tc.tile_pool(name="pp", bufs=2))
        dp = ctx.enter_context(tc.tile_pool(name="dp", bufs=1))
        ap = ctx.enter_context(tc.tile_pool(name="ap", bufs=1))
        acc = ctx.enter_context(tc.tile_pool(name="acc", bufs=1))
        ps = ctx.enter_context(tc.psum_pool(name="ps", bufs=1))

        dscr = dp.tile([P, CRMAX], fp16)   # DVE R-scratch
        fp8 = mybir.dt.float8e4
        ascr = ap.tile([P, CRMAX], fp8)    # ACT R-scratch (dtype-agnostic rate)
        ones = acc.tile([P, 1], fp16)
        nc.vector.memset(ones[:], 1.0)

        NR = len(R_CHUNKS)
        zbR = acc.tile([P, NR], fp32)
        zR = acc.tile([P, NR], fp32)
        bR = acc.tile([P, NR], fp32)
        zzR = acc.tile([P, NR], fp32)
        bbR = acc.tile([P, NR], fp32)
        stats = acc.tile([P, 6], fp32)

        psum = [ps.tile([1, MMW], fp32, name=f"psum{s}")
                for s in range(NSTREAM)]
        mm_idx = [0] * NSTREAM

        def mm(s, src_ap):
            i = mm_idx[s]
            nc.tensor.matmul(psum[s][0:1, :], ones[:, 0:1], src_ap,
                             start=(i == 0), stop=(i == NMM_STREAM - 1))
            mm_idx[s] += 1

        offR = 0
        offT = X
        for kind, idx in SCHED:
            if kind == "R":
                cr = R_CHUNKS[idx]
                zr = zrp.tile([P, cr], fp16, name="zr")
                nc.sync.dma_start(zr[:], z_ext[:, offR:offR + cr])
                br = brp.tile([P, cr], fp16, name="br")
                nc.sync.dma_start(br[:], b_ext[:, offR:offR + cr])
                offR += cr
                rnd = idx
                nc.vector.scalar_tensor_tensor(
                    out=dscr[:, :cr], in0=zr[:], scalar=1.0, in1=br[:],
                    op0=ALU.mult, op1=ALU.mult,
                    accum_out=zbR[:, rnd:rnd + 1])
                nc.scalar.activation(out=ascr[:, :cr], in_=zr[:],
                                     func=AF.Square,
                                     accum_out=zzR[:, rnd:rnd + 1])
                nc.scalar.activation(out=ascr[:, :cr], in_=zr[:],
                                     func=AF.Copy,
                                     accum_out=zR[:, rnd:rnd + 1])
                nc.scalar.activation(out=ascr[:, :cr], in_=br[:],
                                     func=AF.Square,
                                     accum_out=bbR[:, rnd:rnd + 1])
                nc.scalar.activation(out=ascr[:, :cr], in_=br[:],
                                     func=AF.Copy,
                                     accum_out=bR[:, rnd:rnd + 1])
            else:
                cts = T_SUBS[idx]
                zt = ztp.tile([P, cts], fp16, name="zt")
                nc.sync.dma_start(zt[:], z_ext[:, offT:offT + cts])
                bt = btp.tile([P, cts], fp16, name="bt")
                nc.sync.dma_start(bt[:], b_ext[:, offT:offT + cts])
                offT += cts

                for blk in range(cts // MMW):
                    sl = slice(blk * MMW, (blk + 1) * MMW)
                    mm(0, zt[:, sl])
                    mm(1, bt[:, sl])

                pzz = pp.tile([P, cts], fp16, name="pzz")
                nc.vector.tensor_tensor(out=pzz[:], in0=zt[:], in1=zt[:],
                                        op=ALU.mult)
                pzb = pp.tile([P, cts], fp16, name="pzb")
                nc.vector.tensor_tensor(out=pzb[:], in0=zt[:], in1=bt[:],
                                        op=ALU.mult)
                pbb = pp.tile([P, cts], fp16, name="pbb")
                nc.vector.tensor_tensor(out=pbb[:], in0=bt[:], in1=bt[:],
                                        op=ALU.mult)

                for blk in range(cts // MMW):
                    sl = slice(blk * MMW, (blk + 1) * MMW)
                    mm(3, pzz[:, sl])
                    mm(2, pzb[:, sl])
                    mm(4, pbb[:, sl])

        # R stats cols: [zb, z, b, zz, bb]
        nc.vector.tensor_reduce(out=stats[:, 0:1], in_=zbR[:], axis=AX.X, op=ALU.add)
        nc.vector.tensor_reduce(out=stats[:, 1:2], in_=zR[:], axis=AX.X, op=ALU.add)
        nc.vector.tensor_reduce(out=stats[:, 2:3], in_=bR[:], axis=AX.X, op=ALU.add)
        nc.vector.tensor_reduce(out=stats[:, 3:4], in_=zzR[:], axis=AX.X, op=ALU.add)
        nc.vector.tensor_reduce(out=stats[:, 4:5], in_=bbR[:], axis=AX.X, op=ALU.add)
        nc.sync.dma_start(out_ext[:], stats[:])
        tstats = acc.tile([1, NSTREAM * MMW], fp32)
        for s in range(NSTREAM):
            dst = tstats[0:1, s * MMW:(s + 1) * MMW]
            if s % 2 == 1:
                nc.vector.tensor_copy(dst, psum[s][0:1, :])
            else:
                nc.scalar.activation(out=dst, in_=psum[s][0:1, :],
                                     func=AF.Copy)
        nc.sync.dma_start(out2_ext[:], tstats[:])

    nc.finalize()
    return nc


def _get_nc():
    global _NC
    if _NC is None:
        _NC = _build_nc()
    return _NC


def _pack(x):
    # x: [RPC, D] fp16 row block -> [P, DPP]:
    #  cols [0:X]   R layout: partition q*RPC+r holds quarter q of row r's
    #               first D_R elements
    #  cols [X:DPP] T layout: partition p holds x[r, D_R + k*128 + p] at
    #               column X + k*RPC + r
    rpart = x[:, :D_R].reshape(RPC, Q, X).transpose(1, 0, 2).reshape(P, X)
    tpart = x[:, D_R:].reshape(RPC, NBLK, P).transpose(2, 1, 0).reshape(P, Y)
    return np.ascontiguousarray(np.concatenate([rpart, tpart], axis=1))


def kernel(preds, targets, _trace=False):
    preds = np.ascontiguousarray(preds, dtype=np.float32).reshape(N, D)
    targets = np.ascontiguousarray(targets, dtype=np.float32).reshape(N, D)
    preds16 = preds.astype(np.float16)
    targets16 = targets.astype(np.float16)

    in_maps = []
    for c in range(NCORES):
        rows = slice(c * RPC, (c + 1) * RPC)
        in_maps.append({"z": _pack(targets16[rows]),
                        "b": _pack(preds16[rows])})

    res = run_bass_kernel_spmd(_get_nc(), in_maps, list(range(NCORES)),
                               trace=_trace)
    raw = np.stack([res.results[c]["out"] for c in range(NCORES)])  # [8,P,6]
    raw = raw.astype(np.float64)
    S5_R = np.stack([
        raw[..., 1],   # Sz
        raw[..., 2],   # Sb
        raw[..., 3],   # Szz
        raw[..., 4],   # Sbb
        raw[..., 0],   # Szb
    ], axis=-1)
    S_R = S5_R.reshape(NCORES, Q, RPC, 5).sum(axis=1).reshape(N, 5)

    # T-part: psum[s][c] holds partials for row c % RPC
    raw2 = np.stack([res.results[c]["out2"] for c in range(NCORES)])
    raw2 = raw2.astype(np.float64).reshape(NCORES, NSTREAM, MMW // RPC, RPC)
    ST = raw2.sum(axis=2)  # [NCORES, NSTREAM, RPC]; streams: z,b,zb,zz,bb
    S_T = np.stack([ST[:, 0], ST[:, 1], ST[:, 3], ST[:, 4], ST[:, 2]],
                   axis=-1).reshape(N, 5)

    S = S_R + S_T
    Sz, Sb, Szz, Sbb, Szb = (S[:, j] for j in range(5))
    num = Szb - Sz * Sb / D
    vz = Szz - Sz * Sz / D
    vb = Sbb - Sb * Sb / D
    corr = num / (np.sqrt(vz) * np.sqrt(vb) + EPS)
    out = np.array(corr.mean(), dtype=np.float32)
    if _trace:
        return out, res
    return out
